# revision 17
# baseline (speedup 1.0000x reference)
"""DiffusedFarthestAttention Trainium2 kernel (8-core SPMD Bass/Tile).

Decomposition (B=4 batches x 2 halves -> 8 cores; pair (2b, 2b+1) handles batch b):
  Phase 1: to_basis, N-split.  xspec_partial[K,C] = sum_n (evecs[n,:]*mass[n])^T x[n,:]
           over this core's 16384 rows; AllReduce over the pair.  The evfar Gram
           matrix + column sums (for spectral GroupNorm stats) hide under P1's DMA.
  Middle (head-split, 4 heads per core; all 8 programs identical, split lives in
           the DATA): spectral coefs, GroupNorm stats computed spectrally from
           spec1 and the Gram matrix, x_farT via spec1-matmuls, q/k projections
           in transposed layouts, per-head scoresT (32-partition slices of kT/qT)
           -> single 1024-wide exp -> ones-augmented PV giving softmax
           denominators, reciprocal via ACT Exp(-Ln(den)), partition-broadcast
           via PE ones-matmul, out-projection partial (bo/2 each) ->
           zspec_partial; AllReduce #2.
  Phase 3: from_basis, N-split.  out rows = evT_chunk^T @ (coefs_out*zspec*out_w),
           written as bf16 (host casts back to f32; rel-err budget allows).
           evT fully prefetched during the middle; per-partition-contiguous
           row-block layouts keep every DMA at >=2KB descriptors.

Startup: the first SP instructions are pk1 + P1 group-0/1 stream loads so the
PE starts ~5us in; const loads are consolidated (single fused rows broadcast,
single vaug ones fill) and follow.  A tiny warmup AllReduce primes the CC
stream so AR1 doesn't pay the ~12us cold-start trigger delay.

Heavy matmuls run as bfloat16 (P1/P3 streams) or float32r (FP22) elsewhere.
Host-side prep is layout-only (transposes, gathers by far_idx, reshapes, dtype
casts); all arithmetic happens on device.
"""

import numpy as np

import concourse.bass as bass
import concourse.mybir as mybir
import concourse.tile as tile
from concourse import bacc
from concourse.bass_utils import run_bass_kernel_spmd

B, N, K, M = 4, 32768, 128, 1024
C = 256          # C_IN = C_OUT = C_ATT
H, D = 8, 32     # heads, head dim
EPS = 1e-6
P = 128
NH = N // 2      # rows per core
NCH = NH // P    # 128 n-chunks per core
P1G = 8          # n-chunks per P1 group
P3G = 8          # n-chunks per P3 group
HL = H // 2      # heads per core
NMK = M // P
F32 = mybir.dt.float32
F32R = mybir.dt.float32r
BF16 = mybir.dt.bfloat16
FP8 = mybir.dt.float8e4
EVS = 1.0  # (fp8 P1 streams were tried and are numerically untenable)
DT1 = BF16       # phase-1 stream dtype (x, evecs natural)
DT3 = BF16       # phase-3 stream dtype (evT, spec2w)
P3E_BUFS = 16 if DT3 == BF16 else 10
ADD = mybir.AluOpType.add
MULT = mybir.AluOpType.mult
AF = mybir.ActivationFunctionType

# packed f32 param columns (pk1)
_PK1 = dict(massT=(0, NCH), mfarT=(128, NMK), maskh=(136, 2), bkmh=(138, 2),
            gnw=(144, 2), gnb=(146, 2), evals=(148, 1), bq=(149, 1))
PK1_W = 150
# packed f32r matrix columns (pkr): gsum | gbp | ones32 (den bcast) | zeros4
_PKR = dict(gsum=(0, 16), gbp=(16, P))
PKR_W = 180
# packed f32r weight columns (pkw): wq0|wq1|wk0|wk1|wv0|wv1
PKW_W = 2 * P + 2 * P + 2 * C
W33 = HL * (D + 1)


def _build(single=False, phases=(1, 2, 3), reps=1, noar=False):
    """single=True: 1-core variant with AllReduce -> local copy, for TimelineSim."""
    nc = bacc.Bacc("TRN2", target_bir_lowering=False, debug=False,
                   enable_asserts=False, num_devices=1 if single else 8)
    dt = F32
    x_h = nc.dram_tensor("x_h", [NH, C], DT1, kind="ExternalInput").ap()
    ev_h = nc.dram_tensor("ev_h", [NH, K], DT1, kind="ExternalInput").ap()
    evT_h = nc.dram_tensor("evT_h", [K, NH], DT3, kind="ExternalInput").ap()
    evfar = nc.dram_tensor("evfar", [M, K], BF16, kind="ExternalInput").ap()
    evTfar = nc.dram_tensor("evTfar", [K, M], BF16, kind="ExternalInput").ap()
    pk1 = nc.dram_tensor("pk1", [P, PK1_W], F32, kind="ExternalInput").ap()
    pkr = nc.dram_tensor("pkr", [P, PKR_W], F32R, kind="ExternalInput").ap()
    pkw = nc.dram_tensor("pkw", [P, PKW_W], BF16, kind="ExternalInput").ap()
    pkwo = nc.dram_tensor("pkwo", [D, HL * C], BF16, kind="ExternalInput").ap()
    rows = nc.dram_tensor("rows", [1, 5 * C], F32, kind="ExternalInput").ap()
    konst = nc.dram_tensor("konst", [2, NMK * W33], BF16, kind="ExternalInput").ap()
    out_ap = nc.dram_tensor("out", [NH, C], BF16, kind="ExternalOutput").ap()

    RG = [[0, 1], [2, 3], [4, 5], [6, 7]]

    with tile.TileContext(nc) as tc:
        with tc.tile_pool(name="const", bufs=1) as cst, \
             tc.tile_pool(name="mid", bufs=3) as mid, \
             tc.tile_pool(name="p3e", bufs=P3E_BUFS) as p3e, \
             tc.tile_pool(name="dram", bufs=1, space="DRAM") as dram:
            for rep in range(reps):
                # ---- critical-path first: pk1 (massT), then P1 group-0/1 ----
                pk1_t = cst.tile([P, PK1_W], dt, tag="pk1")
                nc.sync.dma_start(pk1_t[:], pk1[:])

                with tc.tile_pool(name="p1x", bufs=3) as p1x, \
                     tc.tile_pool(name="p1e", bufs=3) as p1e, \
                     tc.tile_pool(name="ps1", bufs=1, space="PSUM") as ps1:
                    NPRE = 2
                    et_pre, xt_pre = [], []
                    for g in range(NPRE):
                        et = p1e.tile([P, P1G, K], DT1, tag="e8",
                                      name=f"et_pre{g}")
                        nc.sync.dma_start(
                            et[:], ev_h[g * P1G * P:(g + 1) * P1G * P, :]
                            .rearrange("(p j) k -> p j k", j=P1G))
                        xt = p1x.tile([P, P1G, C], DT1, tag="x8",
                                      name=f"xt_pre{g}")
                        nc.sync.dma_start(
                            xt[:], x_h[g * P1G * P:(g + 1) * P1G * P, :]
                            .rearrange("(p j) c -> p j c", j=P1G))
                        et_pre.append(et)
                        xt_pre.append(xt)

                    # warmup collective: primes the CC stream during P1 so
                    # AR1 doesn't pay the cold-start trigger delay
                    if not (single or noar):
                        wrm_in = dram.tile([1, 2], dt, tag="wrmin")
                        wrm_out = dram.tile([1, 2], dt, tag="wrmout")
                        nc.sync.dma_start(wrm_in[:], rows[0:1, 0:2])
                        nc.gpsimd.collective_compute(
                            "AllReduce", ADD, replica_groups=RG,
                            ins=[wrm_in[:].opt()], outs=[wrm_out[:].opt()])

                    # ---- packed params ----
                    pkr_t = cst.tile([P, PKR_W], F32R, tag="pkr")
                    nc.sync.dma_start(pkr_t[:], pkr[:])
                    pkw_t = cst.tile([P, PKW_W], BF16, tag="pkw")
                    nc.sync.dma_start(pkw_t[:], pkw[:])
                    pkwo_t = cst.tile([D, HL * C], BF16, tag="pkwo")
                    nc.sync.dma_start(pkwo_t[:], pkwo[:])

                    def p1(name):
                        o, w = _PK1[name]
                        return pk1_t[:, o:o + w]
                    massT_t, mfarT_t = p1("massT"), p1("mfarT")
                    maskh_t, bkmh_t = p1("maskh"), p1("bkmh")
                    gnw_t, gnb_t = p1("gnw"), p1("gnb")
                    evals_t, bq_t = p1("evals"), p1("bq")
                    gsum_t = pkr_t[:, 0:16]
                    gbp_t = pkr_t[:, 16:16 + P]
                    wq_t = [pkw_t[:, j * P:(j + 1) * P] for j in range(2)]
                    wk_t = [pkw_t[:, 2 * P + j * P:2 * P + (j + 1) * P]
                            for j in range(2)]
                    wv_t = [pkw_t[:, 4 * P + j * C:4 * P + (j + 1) * C]
                            for j in range(2)]
                    wo_t = [pkwo_t[0:D, h * C:(h + 1) * C] for h in range(HL)]

                    ones32 = cst.tile([P, 32], BF16, tag="ones32")
                    nc.sync.dma_start(ones32[:], konst[0:1, 0:32].to_broadcast([P, 32]))
                    onesr_t = pkr_t[:, 144:176]
                    # fused row params broadcast (one DMA for all 5 rows)
                    rowsb = cst.tile([P, 5 * C], dt, tag="rowsb")
                    nc.sync.dma_start(rowsb[:], rows[0:1, :].to_broadcast([P, 5 * C]))
                    tin_b = rowsb[:, 0:C]
                    tout_b = rowsb[:, C:2 * C]
                    outw_b = rowsb[:, 2 * C:3 * C]
                    bv_b = rowsb[:, 3 * C:3 * C + P]
                    bo_b = rowsb[:, 4 * C:5 * C]
                    nc.vector.tensor_scalar_max(tin_b, tin_b, 1e-8)
                    nc.vector.tensor_scalar_max(tout_b, tout_b, 1e-8)
                    nc.vector.tensor_scalar_max(outw_b, outw_b, 1e-8)

                    # coefs = exp(-evals x t)
                    coef_in = cst.tile([P, C], dt, tag="coefin")
                    nc.vector.tensor_tensor(coef_in[:], evals_t.to_broadcast([P, C]),
                                            tin_b, MULT)
                    nc.scalar.activation(coef_in[:], coef_in[:], AF.Exp, scale=-1.0)
                    coef_out = cst.tile([P, C], dt, tag="coefout")
                    nc.vector.tensor_tensor(coef_out[:], evals_t.to_broadcast([P, C]),
                                            tout_b, MULT)
                    nc.scalar.activation(coef_out[:], coef_out[:], AF.Exp, scale=-1.0)

                    if 2 in phases:
                        # out-projection accumulators (one slab), bo-initialized
                        atall = cst.tile([P, NMK * C], BF16, tag="atall")
                        for mc in range(NMK):
                            nc.vector.tensor_copy(
                                out=atall[:, mc * C:(mc + 1) * C], in_=bo_b)
                        # vaug: one tile, single ones-fill DMA; v blocks written later
                        vaug = cst.tile([P, NMK * W33], BF16, tag="vaug")
                        nc.sync.dma_start(
                            vaug[:], konst[0:1, 0:NMK * W33]
                            .to_broadcast([P, NMK * W33]))
                        ef_all = cst.tile([P, NMK, K], BF16, tag="efall")
                        nc.sync.dma_start(
                            ef_all[:], evfar[:, :].rearrange("(m p) k -> p m k", p=P))
                        evTfar_t = cst.tile([K, M], BF16, tag="evTfar")
                        nc.sync.dma_start(evTfar_t[:], evTfar[:])
                        gram_ps = ps1.tile([K, K], dt, tag="gram")
                        s_ps = ps1.tile([K, 2], dt, tag="sps")

                    # =============== PHASE 1: to_basis (N-split) ===============
                    xspec_ps = ps1.tile([K, C], dt, tag="xspec")
                    ng = NCH // P1G
                    for g in range(ng):
                        if g < NPRE:
                            et, xt = et_pre[g], xt_pre[g]
                        else:
                            et = p1e.tile([P, P1G, K], DT1, tag="e8")
                            nc.sync.dma_start(
                                et[:], ev_h[g * P1G * P:(g + 1) * P1G * P, :]
                                .rearrange("(p j) k -> p j k", j=P1G))
                            xt = p1x.tile([P, P1G, C], DT1, tag="x8")
                            nc.sync.dma_start(
                                xt[:], x_h[g * P1G * P:(g + 1) * P1G * P, :]
                                .rearrange("(p j) c -> p j c", j=P1G))
                        for j in range(P1G):
                            cix = g * P1G + j
                            nc.vector.tensor_tensor(
                                et[:, j, :], et[:, j, :],
                                massT_t[:, cix:cix + 1].to_broadcast([P, K]), MULT)
                        for j in range(P1G):
                            nc.tensor.matmul(xspec_ps[:], et[:, j, :], xt[:, j, :],
                                             start=(g == 0 and j == 0),
                                             stop=(g == ng - 1 and j == P1G - 1))
                        if g == 4 and 2 in phases:
                            # Gram + column sums, enqueued mid-P1 so they don't
                            # gate the first P1 matmul on the ef_all DMA
                            for mc in range(NMK):
                                nc.tensor.matmul(gram_ps[:], ef_all[:, mc, :],
                                                 ef_all[:, mc, :],
                                                 start=(mc == 0), stop=(mc == NMK - 1))
                                nc.tensor.matmul(s_ps[:], ef_all[:, mc, :],
                                                 ones32[:, 0:2],
                                                 start=(mc == 0), stop=(mc == NMK - 1))
                            for mc in range(NMK):
                                nc.vector.tensor_scalar_mul(
                                    ef_all[:, mc, :], ef_all[:, mc, :],
                                    mfarT_t[:, mc:mc + 1])
                            gram_sb = cst.tile([K, K], BF16, tag="gram_sb")
                            nc.scalar.copy(gram_sb[:], gram_ps[:])
                            s_sb = cst.tile([K, 2], BF16, tag="s_sb")
                            nc.vector.tensor_copy(out=s_sb[:], in_=s_ps[:])
                    xspec_sb = cst.tile([K, C], BF16, tag="xspec_sb")
                    nc.scalar.copy(xspec_sb[:], xspec_ps[:])

                # Exchange #1 (pair): AllGather of bf16 xspec partials; summed
                # locally (order-independent, so the SPMD program stays
                # rank-agnostic).  evT prefetch issues right after so the
                # transfers fill the middle phase's otherwise-idle DMA.
                ar1_in = dram.tile([K, C], BF16, tag="ar1in")
                ar1_out = dram.tile([2 * K, C], BF16, tag="ar1out")
                nc.sync.dma_start(ar1_in[:], xspec_sb[:])
                if single or noar:
                    nc.sync.dma_start(ar1_out[0:K, :], ar1_in[:])
                    nc.sync.dma_start(ar1_out[K:2 * K, :], ar1_in[:])
                else:
                    nc.gpsimd.collective_compute(
                        "AllGather", mybir.AluOpType.bypass, replica_groups=RG,
                        ins=[ar1_in[:].opt()], outs=[ar1_out[:].opt()])
                if 3 in phases:
                    p3et = [p3e.tile([K, P3G * P], DT3, tag="evt8", bufs=P3E_BUFS,
                                     name=f"p3et{g}") for g in range(NCH // P3G)]
                    for g in range(P3E_BUFS):
                        nc.sync.dma_start(p3et[g][:],
                                          evT_h[:, g * P3G * P:(g + 1) * P3G * P])

                if 2 in phases:
                    # xfT from the LOCAL xspec partial, during the exchange
                    spec1a = cst.tile([K, C], BF16, tag="spec1a")
                    nc.vector.scalar_tensor_tensor(spec1a[:], coef_in[:], 1.0 / EVS,
                                                   xspec_sb[:], MULT, MULT)
                    xfT = [cst.tile([P, M], BF16, tag=f"xfT{cc}", name=f"xfT{cc}")
                           for cc in range(2)]
                    with tc.tile_pool(name="psa", bufs=2, space="PSUM") as psa:
                        for cc in range(2):
                            for mh in range(2):
                                pxa = psa.tile([P, 512], dt, tag="pxa", bufs=2)
                                nc.tensor.matmul(pxa[:], spec1a[:, cc * P:(cc + 1) * P],
                                                 evTfar_t[:, mh * 512:(mh + 1) * 512],
                                                 start=True, stop=True)
                                nc.vector.tensor_copy(
                                    out=xfT[cc][:, mh * 512:(mh + 1) * 512], in_=pxa[:])

                xs_g = [cst.tile([K, C], BF16, tag=f"xsg{i}", name=f"xs_g{i}")
                        for i in range(2)]
                for i in range(2):
                    nc.sync.dma_start(xs_g[i][:], ar1_out[i * K:(i + 1) * K, :])
                xspec_sum = cst.tile([K, C], dt, tag="xspec_sum")
                nc.vector.tensor_tensor(xspec_sum[:], xs_g[0][:], xs_g[1][:], ADD)
                spec1 = cst.tile([K, C], BF16, tag="spec1")
                nc.vector.scalar_tensor_tensor(spec1[:], coef_in[:], 1.0 / EVS,
                                               xspec_sum[:], MULT, MULT)

                if 2 in phases:
                    # =============== MIDDLE ===============
                    xspec_b = cst.tile([K, C], dt, tag="xspec_b")
                    nc.vector.tensor_sub(xspec_b[:], xspec_sum[:], xspec_sb[:])
                    spec1b = cst.tile([K, C], BF16, tag="spec1b")
                    nc.vector.scalar_tensor_tensor(spec1b[:], coef_in[:], 1.0 / EVS,
                                                   xspec_b[:], MULT, MULT)
                    with tc.tile_pool(name="psm", bufs=1, space="PSUM") as psm:
                        # ---- xfT += peer-partial contribution ----
                        for cc in range(2):
                            for mh in range(2):
                                pxb = psm.tile([P, 512], dt, tag="psc2", bufs=2)
                                nc.tensor.matmul(pxb[:], spec1b[:, cc * P:(cc + 1) * P],
                                                 evTfar_t[:, mh * 512:(mh + 1) * 512],
                                                 start=True, stop=True)
                                nc.vector.tensor_tensor(
                                    xfT[cc][:, mh * 512:(mh + 1) * 512],
                                    xfT[cc][:, mh * 512:(mh + 1) * 512], pxb[:], ADD)
                        # ---- spectral GN stats ----
                        t1 = psm.tile([K, C], dt, tag="mm256", bufs=2)
                        nc.tensor.matmul(t1[:], gram_sb[:], spec1[:],
                                         start=True, stop=True)
                        sq = cst.tile([K, C], BF16, tag="sq")
                        nc.vector.tensor_tensor(sq[:], spec1[:], t1[:], MULT)
                        # fp32r matmuls need even free dims: stats come out as
                        # duplicated column pairs, compacted below
                        stat_ps = psm.tile([P, 8], dt, tag="mm256", bufs=2)
                        for cc in range(2):
                            nc.tensor.matmul(stat_ps[:, 2 * cc:2 * cc + 2],
                                             spec1[:, cc * P:(cc + 1) * P], s_sb[:],
                                             start=True, stop=True)
                            nc.tensor.matmul(stat_ps[:, 4 + 2 * cc:6 + 2 * cc],
                                             sq[:, cc * P:(cc + 1) * P],
                                             ones32[:, 0:2],
                                             start=True, stop=True)
                        stat_mq = cst.tile([P, 8], F32R, tag="statmq")
                        nc.vector.tensor_copy(out=stat_mq[:], in_=stat_ps[:])
                        pg = psm.tile([16, 8], dt, tag="mm256", bufs=2)
                        nc.tensor.matmul(pg[:], gsum_t, stat_mq[:], start=True, stop=True)
                        inv = 1.0 / (M * 8)
                        mu = cst.tile([16, 2], dt, tag="mu")
                        nc.vector.tensor_scalar_mul(mu[:, 0:1], pg[:, 0:1], inv)
                        nc.vector.tensor_scalar_mul(mu[:, 1:2], pg[:, 2:3], inv)
                        ms = cst.tile([16, 2], dt, tag="ms")
                        nc.vector.tensor_scalar_mul(ms[:, 0:1], pg[:, 4:5], inv)
                        nc.vector.tensor_scalar_mul(ms[:, 1:2], pg[:, 6:7], inv)
                        var = cst.tile([16, 2], dt, tag="var")
                        nc.vector.tensor_tensor(var[:], mu[:], mu[:], MULT)
                        nc.vector.tensor_sub(var[:], ms[:], var[:])
                        nc.vector.tensor_scalar_add(var[:], var[:], EPS)
                        std = cst.tile([16, 2], dt, tag="std")
                        nc.scalar.activation(std[:], var[:], AF.Sqrt)
                        rstd = cst.tile([16, 2], dt, tag="rstd")
                        nc.vector.reciprocal(rstd[:], std[:])
                        stats_sb = cst.tile([P, 4], F32R, tag="stats")
                        nc.vector.tensor_copy(out=stats_sb[:], in_=pkr_t[:, 176:180])
                        nc.vector.tensor_copy(out=stats_sb[0:16, 0:2], in_=mu[:])
                        nc.vector.tensor_copy(out=stats_sb[0:16, 2:4], in_=rstd[:])
                        pbc = psm.tile([P, 4], dt, tag="mm256", bufs=2)
                        nc.tensor.matmul(pbc[:], gbp_t, stats_sb[:], start=True, stop=True)
                        A = cst.tile([P, 2], dt, tag="gnA")
                        nc.vector.tensor_tensor(A[:], pbc[:, 2:4], gnw_t, MULT)
                        Bt = cst.tile([P, 2], dt, tag="gnB")
                        nc.vector.tensor_tensor(Bt[:], pbc[:, 0:2], A[:], MULT)
                        nc.vector.tensor_sub(Bt[:], gnb_t, Bt[:])

                        # ---- GN affine on xfT ----
                        for cc in range(2):
                            nc.vector.scalar_tensor_tensor(
                                xfT[cc][:], xfT[cc][:], A[:, cc:cc + 1],
                                Bt[:, cc:cc + 1].to_broadcast([P, M]), MULT, ADD)

                        # ---- qT / kT projections ----
                        # kT in two half-masked copies (even/odd heads kept):
                        # scores contract over 64-row windows at bases 0/64,
                        # where the other head's kT rows are zero
                        qT = cst.tile([P, M], BF16, tag="qT")
                        kTz = [cst.tile([P, M], BF16, tag=f"kTz{z}",
                                        name=f"kTz{z}") for z in range(2)]
                        for mh in range(2):
                            pq = psm.tile([P, 512], dt, tag="psc2", bufs=2)
                            pk = psm.tile([P, 512], dt, tag="psc2", bufs=2)
                            for cin in range(2):
                                nc.tensor.matmul(pq[:], wq_t[cin],
                                                 xfT[cin][:, mh * 512:(mh + 1) * 512],
                                                 start=(cin == 0), stop=(cin == 1))
                            for cin in range(2):
                                nc.tensor.matmul(pk[:], wk_t[cin],
                                                 xfT[cin][:, mh * 512:(mh + 1) * 512],
                                                 start=(cin == 0), stop=(cin == 1))
                            nc.vector.tensor_tensor(qT[:, mh * 512:(mh + 1) * 512], pq[:],
                                                    bq_t.to_broadcast([P, 512]), ADD)
                            for z in range(2):
                                nc.vector.scalar_tensor_tensor(
                                    kTz[z][:, mh * 512:(mh + 1) * 512], pk[:],
                                    maskh_t[:, z:z + 1],
                                    bkmh_t[:, z:z + 1].to_broadcast([P, 512]),
                                    MULT, ADD)

                        # ---- v (natural, my-half cols first) into vaug blocks ----
                        for mc in range(NMK):
                            pv = psm.tile([P, C], dt, tag="mm256", bufs=2)
                            for cin in range(2):
                                nc.tensor.matmul(pv[:], xfT[cin][:, mc * P:(mc + 1) * P],
                                                 wv_t[cin],
                                                 start=(cin == 0), stop=(cin == 1))
                            for h in range(HL):
                                nc.vector.tensor_add(
                                    vaug[:, mc * W33 + h * (D + 1):
                                            mc * W33 + h * (D + 1) + D],
                                    pv[:, h * D:(h + 1) * D],
                                    bv_b[:, h * D:(h + 1) * D])

                        # ---- attention: scoresT -> 1024-wide exp -> PV ----
                        oTh = [cst.tile([D, M], BF16, tag=f"oTh{h}",
                                        name=f"oTh{h}") for h in range(HL)]
                        scl = 1.0 / np.sqrt(D)
                        for h in range(HL):
                            hw = slice((h // 2) * 64, (h // 2) * 64 + 64)
                            kTh = kTz[h % 2]
                            po = [psm.tile([D + 1, 512], dt, tag="po", bufs=2,
                                           name=f"po{h}_{q}") for q in range(2)]
                            pts = [None] * NMK
                            for mkc in range(NMK):
                                psc = psm.tile([P, 1024], dt, tag="psc2", bufs=2)
                                for q in range(2):
                                    nc.tensor.matmul(
                                        psc[:, q * 512:(q + 1) * 512],
                                        kTh[hw, mkc * P:(mkc + 1) * P],
                                        qT[hw, q * 512:(q + 1) * 512],
                                        start=True, stop=True)
                                pt = mid.tile([P, 1024], BF16, tag="ptile", bufs=4,
                                              name=f"pt{mkc}")
                                # two 512-wide exps: one ACT read must stay
                                # within a single PSUM bank
                                for q in range(2):
                                    nc.scalar.activation(pt[:, q * 512:(q + 1) * 512],
                                                         psc[:, q * 512:(q + 1) * 512],
                                                         AF.Exp, scale=scl)
                                pts[mkc] = pt
                                if mkc > 0:
                                    for q in range(2):
                                        nc.tensor.matmul(
                                            po[q][:],
                                            vaug[:, (mkc - 1) * W33 + h * (D + 1):
                                                    (mkc - 1) * W33 + (h + 1) * (D + 1)],
                                            pts[mkc - 1][:, q * 512:(q + 1) * 512],
                                            start=(mkc - 1 == 0), stop=False)
                            for q in range(2):
                                nc.tensor.matmul(
                                    po[q][:],
                                    vaug[:, (NMK - 1) * W33 + h * (D + 1):
                                            (NMK - 1) * W33 + (h + 1) * (D + 1)],
                                    pts[NMK - 1][:, q * 512:(q + 1) * 512],
                                    start=False, stop=True)
                            # 1/den via ACT Exp(-Ln(den)); partition-broadcast
                            # via PE ones-matmul (no DRAM bounce, no DVE recip)
                            den1 = mid.tile([D + 1, M], F32R, tag="den1", bufs=2,
                                            name=f"den1_{h}")
                            db_sb = mid.tile([D, M], F32R, tag="dbsb", bufs=2,
                                             name=f"dbsb{h}")
                            for q in range(2):
                                sl = slice(q * 512, (q + 1) * 512)
                                nc.scalar.activation(den1[D:D + 1, sl],
                                                     po[q][D:D + 1, :], AF.Ln)
                                nc.scalar.activation(den1[D:D + 1, sl],
                                                     den1[D:D + 1, sl],
                                                     AF.Exp, scale=-1.0)
                                db_ps = psm.tile([D, 512], dt, tag="mm256", bufs=2,
                                                 name=f"dbps{h}_{q}")
                                nc.tensor.matmul(db_ps[:], onesr_t[D:D + 1, 0:D],
                                                 den1[D:D + 1, sl],
                                                 start=True, stop=True)
                                nc.vector.tensor_copy(out=db_sb[:, sl], in_=db_ps[:])
                                nc.vector.tensor_tensor(oTh[h][:, sl],
                                                        po[q][0:D, :],
                                                        db_sb[:, sl], MULT)
                                for mc in range(q * 4, q * 4 + 4):
                                    pa = psm.tile([P, C], dt, tag="mm256", bufs=2,
                                                  name=f"pa{h}_{mc}")
                                    nc.tensor.matmul(pa[:],
                                                     oTh[h][:, mc * P:(mc + 1) * P],
                                                     wo_t[h], start=True, stop=True)
                                    nc.vector.tensor_tensor(
                                        atall[:, mc * C:(mc + 1) * C],
                                        atall[:, mc * C:(mc + 1) * C], pa[:], ADD)

                        # ---- zspec partial (out-proj already accumulated) ----
                        zspec_ps = psm.tile([K, C], dt, tag="po", bufs=2)
                        for mc in range(NMK):
                            nc.tensor.matmul(zspec_ps[:], ef_all[:, mc, :],
                                             atall[:, mc * C:(mc + 1) * C],
                                             start=(mc == 0), stop=(mc == NMK - 1))
                        zspec_sb = cst.tile([K, C], BF16, tag="zspec_sb")
                        nc.scalar.copy(zspec_sb[:], zspec_ps[:])

                    # Exchange #2 (pair): AllGather bf16 zspec partials, local sum
                    ar2_in = dram.tile([K, C], BF16, tag="ar2in")
                    ar2_out = dram.tile([2 * K, C], BF16, tag="ar2out")
                    nc.sync.dma_start(ar2_in[:], zspec_sb[:])
                    if single or noar:
                        nc.sync.dma_start(ar2_out[0:K, :], ar2_in[:])
                        nc.sync.dma_start(ar2_out[K:2 * K, :], ar2_in[:])
                    else:
                        nc.gpsimd.collective_compute(
                            "AllGather", mybir.AluOpType.bypass, replica_groups=RG,
                            ins=[ar2_in[:].opt()], outs=[ar2_out[:].opt()])
                    zs_g = [cst.tile([K, C], BF16, tag=f"zsg{i}", name=f"zs_g{i}")
                            for i in range(2)]
                    for i in range(2):
                        nc.sync.dma_start(zs_g[i][:], ar2_out[i * K:(i + 1) * K, :])
                    zspec_sum = cst.tile([K, C], dt, tag="zspec_sum")
                    nc.vector.tensor_tensor(zspec_sum[:], zs_g[0][:], zs_g[1][:], ADD)

                    spec2 = cst.tile([K, C], DT3, tag="spec2")
                    nc.vector.tensor_tensor(spec2[:], coef_out[:], zspec_sum[:], MULT)
                    nc.vector.tensor_tensor(spec2[:], spec2[:], outw_b, MULT)

                if 3 not in phases:
                    nc.sync.dma_start(out_ap[0:P, :], xspec_sum[:])
                if 3 in phases:
                    # =============== PHASE 3: from_basis (N-split) ===============
                    with tc.tile_pool(name="p3o", bufs=3) as p3o, \
                         tc.tile_pool(name="ps3", bufs=6, space="PSUM") as ps3:
                        ng = NCH // P3G
                        for g in range(ng):
                            if g >= P3E_BUFS:
                                nc.sync.dma_start(
                                    p3et[g][:],
                                    evT_h[:, g * P3G * P:(g + 1) * P3G * P])
                            et = p3et[g]
                            ot = p3o.tile([P, P3G, C], BF16, tag="out8")
                            for j in range(P3G):
                                pp = ps3.tile([P, C], dt, tag="p3")
                                nc.tensor.matmul(pp[:], et[:, j * P:(j + 1) * P],
                                                 spec2[:], start=True, stop=True)
                                if j % 2 == 0:
                                    nc.vector.tensor_copy(out=ot[:, j, :], in_=pp[:])
                                else:
                                    nc.scalar.copy(ot[:, j, :], pp[:])
                            nc.sync.dma_start(
                                out_ap[g * P3G * P:(g + 1) * P3G * P, :]
                                .rearrange("(p j) c -> p j c", j=P3G),
                                ot[:])

    nc.compile()
    return nc


_PROG = None


def _get_prog():
    global _PROG
    if _PROG is None:
        _PROG = _build()
    return _PROG


def make_in_maps(x, mass, evals, evecs, far_idx, diff_in_t, diff_out_t, gn_w, gn_b,
                 Wq, bq, Wk, bk, Wv, bv, Wo, bo, out_w):
    """Host-side (layout-only) prep of the 8 per-core input dicts."""
    import ml_dtypes
    f32 = np.float32
    np1 = ml_dtypes.bfloat16 if DT1 == BF16 else f32
    np3 = ml_dtypes.bfloat16 if DT3 == BF16 else f32
    asf = lambda a: np.ascontiguousarray(a, dtype=f32)
    x = np.asarray(x, dtype=f32)
    mass = np.asarray(mass, dtype=f32)
    evals = np.asarray(evals, dtype=f32)
    evecs = np.asarray(evecs, dtype=f32)
    far_idx = np.asarray(far_idx)
    gsum_m = np.zeros((P, 16), f32)
    gsum_m[np.arange(P), np.arange(P) // 8] = 1.0
    gbp_m = np.zeros((P, P), f32)
    gbp_m[np.arange(P) // 8, np.arange(P)] = 1.0
    konst_m = np.stack([np.ones(NMK * W33, np3), np.zeros(NMK * W33, np3)])
    in_maps = []
    for core in range(8):
        b, half = core // 2, core % 2
        rs = slice(half * NH, (half + 1) * NH)
        hc = slice(half * P, (half + 1) * P)        # my C_ATT columns / heads
        oc = slice((1 - half) * P, (2 - half) * P)  # partner's columns
        fi = far_idx[b]
        ev_far = evecs[b][fi]                       # [M, K]
        pk1_m = np.zeros((P, PK1_W), f32)
        pk1_m[:, 0:NCH] = (mass[b, rs].reshape(NCH // P1G, P, P1G)
                           .transpose(1, 0, 2).reshape(P, NCH))
        pk1_m[:, 128:128 + NMK] = mass[b][fi].reshape(NMK, P).T
        maskh = ((np.arange(P)[:, None] // D) % 2 == np.arange(2)[None, :]) \
            .astype(f32)
        pk1_m[:, 136:138] = maskh
        pk1_m[:, 138:140] = maskh * np.asarray(bk)[hc][:, None]
        pk1_m[:, 144:146] = np.asarray(gn_w).reshape(2, P).T
        pk1_m[:, 146:148] = np.asarray(gn_b).reshape(2, P).T
        pk1_m[:, 148] = evals[b]
        pk1_m[:, 149] = np.asarray(bq)[hc]
        pkr_m = np.concatenate([gsum_m, gbp_m, np.ones((P, 32), f32),
                                np.zeros((P, 4), f32)], axis=1)
        pkw_m = np.concatenate(
            [np.asarray(Wq)[:, hc].reshape(2, P, P).transpose(1, 0, 2).reshape(P, 2 * P),
             np.asarray(Wk)[:, hc].reshape(2, P, P).transpose(1, 0, 2).reshape(P, 2 * P),
             np.concatenate([np.asarray(Wv)[:, hc], np.asarray(Wv)[:, oc]], axis=1)
             .reshape(2, P, C).transpose(1, 0, 2).reshape(P, 2 * C)], axis=1)
        pkwo_m = np.asarray(Wo)[hc].reshape(HL, D, C).transpose(1, 0, 2) \
            .reshape(D, HL * C)
        rows_m = np.zeros((1, 5 * C), f32)
        rows_m[0, 0:C] = np.asarray(diff_in_t)
        rows_m[0, C:2 * C] = np.asarray(diff_out_t)
        rows_m[0, 2 * C:3 * C] = np.asarray(out_w)
        rows_m[0, 3 * C:3 * C + P] = np.asarray(bv)[hc]
        rows_m[0, 4 * C:5 * C] = 0.5 * np.asarray(bo)
        m = {
            "x_h": np.ascontiguousarray(x[b, rs], dtype=np1),
            "ev_h": np.ascontiguousarray(evecs[b, rs], dtype=np1),
            "evT_h": np.ascontiguousarray(
                evecs[b, rs].T.reshape(K, NCH // P3G, P, P3G)
                .transpose(0, 1, 3, 2).reshape(K, NH), dtype=np3),
            "evfar": np.ascontiguousarray(ev_far, dtype=np3),
            "evTfar": np.ascontiguousarray(ev_far.T, dtype=np3),
            "pk1": pk1_m,
            "pkr": pkr_m,
            "pkw": np.ascontiguousarray(pkw_m, dtype=np3),
            "pkwo": np.ascontiguousarray(pkwo_m, dtype=np3),
            "rows": rows_m,
            "konst": konst_m,
        }
        in_maps.append(m)
    return in_maps


def kernel(**inputs):
    nc = _get_prog()
    in_maps = make_in_maps(**inputs)
    res = run_bass_kernel_spmd(nc, in_maps, core_ids=list(range(8)))
    out = np.empty((B, N, C), np.float32)
    for core in range(8):
        b, half = core // 2, core % 2
        out[b, half * NH:(half + 1) * NH] = \
            np.asarray(res.results[core]["out"]).astype(np.float32)
    return out
